# revision 30
# baseline (speedup 1.0000x reference)
"""GQA multi-head attention (B=2, S=2048, D=2048, HQ=16, HKV=4, DK=128) with
RoPE + causal softmax + output projection, sharded over 8 NeuronCores as
(batch x kv-head-group): core c handles batch c//4, kv head c%4 (4 query
heads). w_q/w_kv column-sharded, fc row-sharded; partial fc outputs are
summed on the host (the "all-reduce").

Schedule: K/V projections run first as the xT-DMA-paced pass (their PE
appetite matches chunk arrival), Q projection halves then run from
SBUF-resident xT at full speed. Attention processes query blocks in
descending size with the previous block's fc matmuls interleaved.
"""

import sys

for _p in ("/opt/trn_rl_repo", "/root/.axon_site", "/root/.axon_site/_ro/trn_rl_repo"):
    if _p not in sys.path:
        sys.path.insert(0, _p)

import numpy as np

import concourse.bass as bass
import concourse.mybir as mybir
import concourse.tile as tile
from concourse import bacc
from concourse.bass_utils import run_bass_kernel_spmd

F32 = mybir.dt.float32
F16 = mybir.dt.float16

B, S, D = 2, 2048, 2048
HKV, NREP, DK = 4, 4, 128
HG = NREP  # query heads per core
KC = D // 128  # contraction chunks
SQC = S // 512  # 512-wide query column chunks
SCALE = float(1.0 / np.sqrt(DK))

_COMPILED = None


def _build():
    nc = bacc.Bacc(None, target_bir_lowering=False, debug=False)

    xT = nc.dram_tensor("xT", [D, S], F16, kind="ExternalInput")
    wq = nc.dram_tensor("wq", [D, HG * DK], F16, kind="ExternalInput")
    wk = nc.dram_tensor("wk", [D, DK], F16, kind="ExternalInput")
    wv = nc.dram_tensor("wv", [D, DK], F16, kind="ExternalInput")
    fcw = nc.dram_tensor("fcw", [HG * DK, D], F16, kind="ExternalInput")
    cosT = nc.dram_tensor("cosT", [128, S], F16, kind="ExternalInput")
    sinT = nc.dram_tensor("sinT", [128, S], F16, kind="ExternalInput")
    tri = nc.dram_tensor("tri", [128, 128], F16, kind="ExternalInput")
    onesc = nc.dram_tensor("onesc", [128, 1], F16, kind="ExternalInput")
    iden = nc.dram_tensor("iden", [128, 128], F16, kind="ExternalInput")
    out = nc.dram_tensor("out", [S, D], F32, kind="ExternalOutput")

    with tile.TileContext(nc) as tc:
        with tc.tile_pool(name="persist", bufs=1) as persist:
            # attention-phase residents
            qt_sb = persist.tile([128, HG, S], F16)  # Q^T, rope'd, per head
            kt_sb = persist.tile([128, S], F16)  # K^T rope'd
            v_sb = persist.tile([128, KC, DK], F16)  # V  [sk, dk] chunks
            ctxT = persist.tile([128, HG, S], F16)  # (softmax @ V)^T per head
            cos_sb = persist.tile([128, S], F16)
            sin_sb = persist.tile([128, S], F16)
            tri_sb = persist.tile([128, 128], F16)
            ones_sb = persist.tile([128, 1], F16)
            iden_sb = persist.tile([128, 128], F16)
            fcw_sb = persist.tile([128, HG, D], F16)

            # pools shared across all phases (no release/realloc barriers)
            ps8 = tc.alloc_tile_pool(name="ps8", bufs=8, space="PSUM")
            es_pool = tc.alloc_tile_pool(name="es_pool", bufs=5)
            nrm_pool = tc.alloc_tile_pool(name="nrm_pool", bufs=3)
            # rope staging outlives phase 1 (mh2/mh3 rope runs during early
            # attention)
            ropep = tc.alloc_tile_pool(name="ropep", bufs=2)

            with tc.tile_pool(name="p1sb", bufs=1) as p1sb:
                xt_sb = p1sb.tile([128, KC, S], F16)
                wq_sb = p1sb.tile([128, KC, HG * DK], F16)
                wk_sb = p1sb.tile([128, KC, DK], F16)
                wv_sb = p1sb.tile([128, KC, DK], F16)
                vt_sb = p1sb.tile([128, S], F16)

                # DMA priority order. sync ring: the 16 xT chunks (the
                # critical stream — K/V projection consumes them at DMA
                # pace). scalar ring: wk/wv first (needed by the paced
                # pass), then wq quarters (Q passes start after chunk 15),
                # then everything needed later still.
                xr = xT.rearrange("(k p) s -> p k s", p=128)
                for k in range(KC):
                    nc.sync.dma_start(out=xt_sb[:, k, :], in_=xr[:, k, :])
                wkr = wk.rearrange("(k p) m -> p k m", p=128)
                wvr = wv.rearrange("(k p) m -> p k m", p=128)
                wqr = wq.rearrange("(k p) m -> p k m", p=128)
                # interleave so wq quarters land well before the Q passes
                for q4 in range(4):
                    ks = slice(4 * q4, 4 * (q4 + 1))
                    nc.scalar.dma_start(out=wk_sb[:, ks, :], in_=wkr[:, ks, :])
                    nc.scalar.dma_start(out=wv_sb[:, ks, :], in_=wvr[:, ks, :])
                    if q4 >= 1:
                        ks2 = slice(4 * (q4 - 1), 4 * q4)
                        nc.scalar.dma_start(out=wq_sb[:, ks2, :], in_=wqr[:, ks2, :])
                nc.scalar.dma_start(out=wq_sb[:, 12:16, :], in_=wqr[:, 12:16, :])
                nc.scalar.dma_start(out=cos_sb, in_=cosT[:])
                nc.scalar.dma_start(out=sin_sb, in_=sinT[:])
                nc.scalar.dma_start(out=iden_sb, in_=iden[:])
                nc.scalar.dma_start(out=tri_sb, in_=tri[:])
                nc.scalar.dma_start(out=ones_sb, in_=onesc[:])
                nc.scalar.dma_start(out=fcw_sb, in_=fcw.rearrange("(h p) n -> p h n", p=128))

                def rope_ops(dst, tq):
                    # dst/tq: [128, S] fp16; evens in partitions 0:64, odds 64:128.
                    # cos/sin are duplicated across both halves so every
                    # SBUF*SBUF tensor op has equal input base partitions.
                    pe, po = tq[0:64, :], tq[64:128, :]
                    t1 = ropep.tile([64, S], F16, name="t1", tag="t1")
                    t2 = ropep.tile([64, S], F16, name="t2", tag="t2")
                    t3 = ropep.tile([64, S], F16, name="t3", tag="t1")
                    t4 = ropep.tile([64, S], F16, name="t4", tag="t2")
                    M = mybir.AluOpType
                    return [
                        lambda: nc.vector.tensor_tensor(t1, pe, cos_sb[0:64, :], op=M.mult),
                        lambda: nc.vector.tensor_tensor(t2, po, sin_sb[64:128, :], op=M.mult),
                        lambda: nc.vector.tensor_tensor(dst[0:64, :], t1, t2, op=M.subtract),
                        lambda: nc.vector.tensor_tensor(t3, pe, sin_sb[0:64, :], op=M.mult),
                        lambda: nc.vector.tensor_tensor(t4, po, cos_sb[64:128, :], op=M.mult),
                        lambda: nc.vector.tensor_tensor(dst[64:128, :], t3, t4, op=M.add),
                    ]

                def rope_full(dst, tq):
                    for f in rope_ops(dst, tq):
                        f()

                # ---- K/V projection: the DMA-paced pass (8 PSUM banks) ----
                kaccs = [ps8.tile([128, 512], F32, name="psk", tag="pp")
                         for _ in range(SQC)]
                vaccs = [ps8.tile([128, 512], F32, name="psvt", tag="pp")
                         for _ in range(SQC)]
                for k in range(KC):
                    for qc in range(SQC):
                        nc.tensor.matmul(kaccs[qc], wk_sb[:, k, :],
                                         xt_sb[:, k, qc * 512:(qc + 1) * 512],
                                         start=(k == 0), stop=(k == KC - 1))
                    for sc in range(SQC):
                        nc.tensor.matmul(vaccs[sc], wv_sb[:, k, :],
                                         xt_sb[:, k, sc * 512:(sc + 1) * 512],
                                         start=(k == 0), stop=(k == KC - 1))
                # drain split across ACT and gpsimd so PE isn't gated on one
                # engine's serial copies
                tk = ropep.tile([128, S], F16, name="tk", tag="tq")
                for qc in range(SQC):
                    dst = tk[:, qc * 512:(qc + 1) * 512]
                    if qc % 2 == 0:
                        nc.scalar.copy(dst, kaccs[qc])
                    else:
                        nc.vector.tensor_copy(dst, kaccs[qc])
                for sc in range(SQC):
                    dst = vt_sb[:, sc * 512:(sc + 1) * 512]
                    if sc % 2 == 0:
                        nc.scalar.copy(dst, vaccs[sc])
                    else:
                        nc.vector.tensor_copy(dst, vaccs[sc])
                rope_full(kt_sb, tk)

                # ---- Q^T = wq^T @ xT (resident), two 8-bank halves ----
                def q_half(half):
                    accs = []
                    for mh in (2 * half, 2 * half + 1):
                        for qc in range(SQC):
                            psq = ps8.tile([128, 512], F32, name="psq", tag="pp")
                            accs.append((mh, qc, psq))
                    for k in range(KC):
                        for mh, qc, psq in accs:
                            nc.tensor.matmul(psq, wq_sb[:, k, mh * 128:(mh + 1) * 128],
                                             xt_sb[:, k, qc * 512:(qc + 1) * 512],
                                             start=(k == 0), stop=(k == KC - 1))
                    tqs = {}
                    for mh in (2 * half, 2 * half + 1):
                        tqs[mh] = ropep.tile([128, S], F16, name="tq", tag="tq")
                    for i, (mh, qc, psq) in enumerate(accs):
                        dst = tqs[mh][:, qc * 512:(qc + 1) * 512]
                        if i % 2 == 0:
                            nc.scalar.copy(dst, psq)
                        else:
                            nc.vector.tensor_copy(dst, psq)
                    thunks = []
                    for mh in (2 * half, 2 * half + 1):
                        thunks.extend(rope_ops(qt_sb[:, mh, :], tqs[mh]))
                    return thunks

                for f in q_half(0):
                    f()

                # V^T -> V PE-transposes slot between the Q halves (banks
                # cycle here anyway); V is only needed at attention time.
                for gq in range(4):
                    psv = ps8.tile([128, 512], F16, name="psv", tag="pp")
                    for vt in range(4):
                        skt = gq * 4 + vt
                        nc.tensor.matmul(psv[:, vt * 128:(vt + 1) * 128],
                                         vt_sb[:, skt * 128:(skt + 1) * 128],
                                         iden_sb, is_transpose=True,
                                         start=True, stop=True)
                    nc.vector.tensor_copy(
                        v_sb[:, gq * 4:(gq + 1) * 4, :].rearrange("p a b -> p (a b)"),
                        psv)

                # mh2/mh3 rope thunks are drizzled into the first attention
                # block instead of lumping 13.6us of DVE work ahead of it
                rope23 = q_half(1)

            # ---------------- phase 2+3: attention with fc interleaved ----------------
            # Query blocks descend (qc=3 first): the deepest softmax pipeline
            # runs first, and each block's fc matmuls interleave into the next
            # block's attention to keep PE dense.
            with tc.tile_pool(name="out_sb", bufs=3) as out_sb:

                def fc_block(sqt, ring=None):
                    ob = out_sb.tile([128, D], F32, name="ob", tag="ob")
                    for nf in range(4):
                        psf = ps8.tile([128, 512], F32, name="psf", tag="pp")
                        for h2 in range(HG):
                            nc.tensor.matmul(psf,
                                             ctxT[:, h2, sqt * 128:(sqt + 1) * 128],
                                             fcw_sb[:, h2, nf * 512:(nf + 1) * 512],
                                             start=(h2 == 0), stop=(h2 == HG - 1))
                        dst = ob[:, nf * 512:(nf + 1) * 512]
                        nc.vector.tensor_copy(dst, psf)
                    (ring or nc.sync).dma_start(out=out[sqt * 128:(sqt + 1) * 128, :], in_=ob)

                prev_qc = None
                for qc in (0, 1, 2, 3):
                    for h in range(HG):
                        if qc == 0:
                            # drizzle the mh2/mh3 rope (12 DVE ops) across the
                            # first block's heads; head h needs thunks
                            # 0..6*(h-1) done, and 3 per head keeps exactly
                            # that schedule
                            for f in rope23[3 * h:3 * (h + 1)]:
                                f()
                        nkc = 4 * (qc + 1)  # causal: sk chunks 0..nkc-1
                        npairs = nkc // 2
                        psc = ps8.tile([128, 512], F32, name="psc", tag="pp")
                        psd = ps8.tile([1, 512], F32, name="psd", tag="pp")
                        qs = qt_sb[:, h, qc * 512:(qc + 1) * 512]
                        es_tiles = [None] * nkc

                        def scores(kc):
                            t = kc - 4 * qc
                            pss = ps8.tile([128, 512], F32, name="pss", tag="pp")
                            es = es_pool.tile([128, 512], F16, name="es", tag="es")
                            z = 128 * t if t > 0 else 0  # dead columns on diag tiles
                            if z:
                                nc.vector.memset(es[:, 0:z], 0.0)
                            nc.tensor.matmul(pss[:, z:512], kt_sb[:, kc * 128:(kc + 1) * 128],
                                             qs[:, z:512], start=True, stop=True)
                            nc.scalar.activation(es[:, z:512], pss[:, z:512],
                                                 mybir.ActivationFunctionType.Exp,
                                                 scale=SCALE)
                            if t >= 0:
                                # only the 128-wide diagonal strip is partial;
                                # columns beyond it are fully alive
                                nc.vector.tensor_tensor(es[:, z:z + 128], es[:, z:z + 128],
                                                        tri_sb,
                                                        op=mybir.AluOpType.mult)
                            es_tiles[kc] = es

                        def accum_pv(kc):
                            nc.tensor.matmul(psc, v_sb[:, kc, :], es_tiles[kc],
                                             start=(kc == 0), stop=(kc == nkc - 1))

                        # softmax denominator: pair adds (+ quad adds when the
                        # block is deep) on DVE, ones-matmul per group on PE,
                        # lagging two groups behind the adds
                        use_quads = nkc >= 8
                        n_group = nkc // 4 if use_quads else npairs
                        pairs = []
                        group = []

                        def accum_den_emit(p):
                            esum = es_pool.tile([128, 512], F16, name="esum", tag="esum")
                            nc.vector.tensor_tensor(esum, es_tiles[2 * p],
                                                    es_tiles[2 * p + 1],
                                                    op=mybir.AluOpType.add)
                            pairs.append(esum)
                            if not use_quads:
                                group.append(esum)
                            elif len(pairs) % 2 == 0:
                                eq = es_pool.tile([128, 512], F16, name="equad", tag="equad")
                                nc.vector.tensor_tensor(eq, pairs[-2], pairs[-1],
                                                        op=mybir.AluOpType.add)
                                group.append(eq)

                        def den_mm(r):
                            nc.tensor.matmul(psd, ones_sb, group[r],
                                             start=(r == 0), stop=(r == n_group - 1))

                        den_issued = 0
                        scores(0)
                        scores(1)
                        for p in range(npairs):
                            if p + 1 < npairs:
                                scores(2 * p + 2)
                                scores(2 * p + 3)
                            accum_pv(2 * p)
                            accum_pv(2 * p + 1)
                            accum_den_emit(p)
                            while den_issued < len(group) - 2:
                                den_mm(den_issued)
                                den_issued += 1
                        while den_issued < n_group:
                            den_mm(den_issued)
                            den_issued += 1

                        rec = nrm_pool.tile([1, 512], F32, name="rec", tag="rec")
                        nc.vector.reciprocal_approx_fast(rec, psd)
                        rb = nrm_pool.tile([128, 512], F32, name="rb", tag="rb")
                        nc.gpsimd.partition_broadcast(rb, rec)
                        nc.vector.tensor_tensor(ctxT[:, h, qc * 512:(qc + 1) * 512],
                                                psc, rb, op=mybir.AluOpType.mult)

                        if prev_qc is not None:
                            fc_block(prev_qc * 4 + h)
                    prev_qc = qc

                # final block's output DMAs split across both rings (the ACT
                # ring is idle by now) so the 4MB tail drains in parallel
                for j in range(4):
                    fc_block(12 + j, ring=(nc.scalar if j % 2 else nc.sync))

            ropep.release()
            nrm_pool.release()
            es_pool.release()
            ps8.release()

    nc.compile()
    return nc


def _get_compiled():
    global _COMPILED
    if _COMPILED is None:
        _COMPILED = _build()
    return _COMPILED


def _prep_inputs(x, w_q, w_kv, fc_w, fc_b, freqs_cos, freqs_sin):
    x = np.asarray(x, dtype=np.float32)
    w_q = np.asarray(w_q, dtype=np.float32)
    w_kv = np.asarray(w_kv, dtype=np.float32)
    fc_w = np.asarray(fc_w, dtype=np.float32)
    freqs_cos = np.asarray(freqs_cos, dtype=np.float32)
    freqs_sin = np.asarray(freqs_sin, dtype=np.float32)

    # rope pair permutation: evens then odds within each head's DK block
    perm = np.concatenate([np.arange(0, DK, 2), np.arange(1, DK, 2)])

    cosT = np.ascontiguousarray(freqs_cos.T).astype(np.float16)  # [64, S]
    sinT = np.ascontiguousarray(freqs_sin.T).astype(np.float16)
    cosT = np.concatenate([cosT, cosT], axis=0)  # duplicate across halves
    sinT = np.concatenate([sinT, sinT], axis=0)

    # tri[i, j] = 1 if i <= j (diagonal-strip causal mask)
    tri = (np.arange(128)[:, None] <= np.arange(128)[None, :]).astype(np.float16)
    onesc = np.ones((128, 1), dtype=np.float16)
    iden = np.eye(128, dtype=np.float16)

    in_maps = []
    for c in range(8):
        b, g = divmod(c, 4)
        xT = np.ascontiguousarray(x[b].T).astype(np.float16)
        wq_g = w_q[:, g * HG * DK:(g + 1) * HG * DK].reshape(D, HG, DK)[:, :, perm]
        wq_g = np.ascontiguousarray(wq_g.reshape(D, HG * DK)).astype(np.float16)
        wk_g = np.ascontiguousarray(w_kv[:, g * DK:(g + 1) * DK][:, perm]).astype(np.float16)
        wv_g = np.ascontiguousarray(w_kv[:, HKV * DK + g * DK:HKV * DK + (g + 1) * DK]).astype(np.float16)
        fcw_g = np.ascontiguousarray(fc_w[g * HG * DK:(g + 1) * HG * DK, :]).astype(np.float16)
        in_maps.append({
            "xT": xT, "wq": wq_g, "wk": wk_g, "wv": wv_g, "fcw": fcw_g,
            "cosT": cosT, "sinT": sinT, "tri": tri, "onesc": onesc,
            "iden": iden,
        })
    return in_maps


_WARMED = False


def kernel_run(trace=False, warmup=True, **inputs):
    global _WARMED
    nc = _get_compiled()
    in_maps = _prep_inputs(**inputs)
    if warmup and not _WARMED:
        # first post-compile execution on a cold device is ~15% slower
        # (table loads / HAM state); do a throwaway run
        run_bass_kernel_spmd(nc, in_maps, core_ids=list(range(8)), trace=False)
        _WARMED = True
    res = run_bass_kernel_spmd(nc, in_maps, core_ids=list(range(8)), trace=trace)
    fc_b = np.asarray(inputs["fc_b"], dtype=np.float32)
    out = np.zeros((B, S, D), dtype=np.float32)
    for c in range(8):
        b = c // 4
        out[b] += res.results[c]["out"]
    out += fc_b[None, None, :]
    return out, res


def kernel(**inputs):
    out, _ = kernel_run(trace=False, **inputs)
    return out


# revision 35
# speedup vs baseline: 1.0127x; 1.0127x over previous
"""GQA multi-head attention (B=2, S=2048, D=2048, HQ=16, HKV=4, DK=128) with
RoPE + causal softmax + output projection, sharded over 8 NeuronCores as
(batch x kv-head-group): core c handles batch c//4, kv head c%4 (4 query
heads). w_q/w_kv column-sharded, fc row-sharded; partial fc outputs are
summed on the host (the "all-reduce").

Schedule: K/V projections run first as the xT-DMA-paced pass (their PE
appetite matches chunk arrival), Q projection halves then run from
SBUF-resident xT at full speed. Attention processes query blocks in
descending size with the previous block's fc matmuls interleaved.
"""

import sys

for _p in ("/opt/trn_rl_repo", "/root/.axon_site", "/root/.axon_site/_ro/trn_rl_repo"):
    if _p not in sys.path:
        sys.path.insert(0, _p)

import numpy as np

import concourse.bass as bass
import concourse.mybir as mybir
import concourse.tile as tile
from concourse import bacc
from concourse.bass_utils import run_bass_kernel_spmd

F32 = mybir.dt.float32
F16 = mybir.dt.float16

B, S, D = 2, 2048, 2048
HKV, NREP, DK = 4, 4, 128
HG = NREP  # query heads per core
KC = D // 128  # contraction chunks
SQC = S // 512  # 512-wide query column chunks
SCALE = float(1.0 / np.sqrt(DK))

_COMPILED = None


def _build():
    nc = bacc.Bacc(None, target_bir_lowering=False, debug=False)

    xT = nc.dram_tensor("xT", [D, S], F16, kind="ExternalInput")
    wq = nc.dram_tensor("wq", [D, HG * DK], F16, kind="ExternalInput")
    wk = nc.dram_tensor("wk", [D, DK], F16, kind="ExternalInput")
    wv = nc.dram_tensor("wv", [D, DK], F16, kind="ExternalInput")
    fcw = nc.dram_tensor("fcw", [HG * DK, D], F16, kind="ExternalInput")
    cosT = nc.dram_tensor("cosT", [128, S], F16, kind="ExternalInput")
    sinT = nc.dram_tensor("sinT", [128, S], F16, kind="ExternalInput")
    tri = nc.dram_tensor("tri", [128, 128], F16, kind="ExternalInput")
    onesc = nc.dram_tensor("onesc", [128, 1], F16, kind="ExternalInput")
    iden = nc.dram_tensor("iden", [128, 128], F16, kind="ExternalInput")
    out = nc.dram_tensor("out", [S, D], F32, kind="ExternalOutput")

    with tile.TileContext(nc) as tc:
        with tc.tile_pool(name="persist", bufs=1) as persist:
            # attention-phase residents
            qt_sb = persist.tile([128, HG, S], F16)  # Q^T, rope'd, per head
            kt_sb = persist.tile([128, S], F16)  # K^T rope'd
            v_sb = persist.tile([128, KC, DK], F16)  # V  [sk, dk] chunks
            ctxT = persist.tile([128, HG, S], F16)  # (softmax @ V)^T per head
            cos_sb = persist.tile([128, S], F16)
            sin_sb = persist.tile([128, S], F16)
            tri_sb = persist.tile([128, 128], F16)
            ones_sb = persist.tile([128, 1], F16)
            iden_sb = persist.tile([128, 128], F16)
            fcw_sb = persist.tile([128, HG, D], F16)

            # pools shared across all phases (no release/realloc barriers)
            ps8 = tc.alloc_tile_pool(name="ps8", bufs=8, space="PSUM")
            es_pool = tc.alloc_tile_pool(name="es_pool", bufs=5)
            nrm_pool = tc.alloc_tile_pool(name="nrm_pool", bufs=3)

            with tc.tile_pool(name="p1sb", bufs=1) as p1sb, \
                 tc.tile_pool(name="p1tmp", bufs=2) as p1tmp:
                xt_sb = p1sb.tile([128, KC, S], F16)
                wq_sb = p1sb.tile([128, KC, HG * DK], F16)
                wk_sb = p1sb.tile([128, KC, DK], F16)
                wv_sb = p1sb.tile([128, KC, DK], F16)
                vt_sb = p1sb.tile([128, S], F16)

                # DMA priority order. sync ring: the 16 xT chunks (the
                # critical stream — K/V projection consumes them at DMA
                # pace). scalar ring: wk/wv first (needed by the paced
                # pass), then wq quarters (Q passes start after chunk 15),
                # then everything needed later still.
                xr = xT.rearrange("(k p) s -> p k s", p=128)
                # chunk 0 lands in two pieces so the first K/V matmuls can
                # start ~1.5us earlier
                nc.sync.dma_start(out=xt_sb[:, 0, 0:1024], in_=xr[:, 0, 0:1024])
                nc.sync.dma_start(out=xt_sb[:, 0, 1024:2048], in_=xr[:, 0, 1024:2048])
                for k in range(1, KC):
                    nc.sync.dma_start(out=xt_sb[:, k, :], in_=xr[:, k, :])
                wkr = wk.rearrange("(k p) m -> p k m", p=128)
                wvr = wv.rearrange("(k p) m -> p k m", p=128)
                wqr = wq.rearrange("(k p) m -> p k m", p=128)
                # interleave so wq quarters land well before the Q passes
                for q4 in range(4):
                    ks = slice(4 * q4, 4 * (q4 + 1))
                    nc.scalar.dma_start(out=wk_sb[:, ks, :], in_=wkr[:, ks, :])
                    nc.scalar.dma_start(out=wv_sb[:, ks, :], in_=wvr[:, ks, :])
                    if q4 >= 1:
                        ks2 = slice(4 * (q4 - 1), 4 * q4)
                        nc.scalar.dma_start(out=wq_sb[:, ks2, :], in_=wqr[:, ks2, :])
                nc.scalar.dma_start(out=wq_sb[:, 12:16, :], in_=wqr[:, 12:16, :])
                nc.scalar.dma_start(out=cos_sb, in_=cosT[:])
                nc.scalar.dma_start(out=sin_sb, in_=sinT[:])
                nc.scalar.dma_start(out=iden_sb, in_=iden[:])
                nc.scalar.dma_start(out=tri_sb, in_=tri[:])
                nc.scalar.dma_start(out=ones_sb, in_=onesc[:])
                nc.scalar.dma_start(out=fcw_sb, in_=fcw.rearrange("(h p) n -> p h n", p=128))

                def rope_full(dst, tq):
                    # dst/tq: [128, S] fp16; evens in partitions 0:64, odds 64:128.
                    # cos/sin are duplicated across both halves so every
                    # SBUF*SBUF tensor op has equal input base partitions.
                    pe, po = tq[0:64, :], tq[64:128, :]
                    t1 = p1tmp.tile([64, S], F16, name="t1", tag="t1")
                    t2 = p1tmp.tile([64, S], F16, name="t2", tag="t2")
                    nc.vector.tensor_tensor(t1, pe, cos_sb[0:64, :], op=mybir.AluOpType.mult)
                    nc.vector.tensor_tensor(t2, po, sin_sb[64:128, :], op=mybir.AluOpType.mult)
                    nc.vector.tensor_tensor(dst[0:64, :], t1, t2, op=mybir.AluOpType.subtract)
                    t3 = p1tmp.tile([64, S], F16, name="t3", tag="t1")
                    t4 = p1tmp.tile([64, S], F16, name="t4", tag="t2")
                    nc.vector.tensor_tensor(t3, pe, sin_sb[0:64, :], op=mybir.AluOpType.mult)
                    nc.vector.tensor_tensor(t4, po, cos_sb[64:128, :], op=mybir.AluOpType.mult)
                    nc.vector.tensor_tensor(dst[64:128, :], t3, t4, op=mybir.AluOpType.add)

                # ---- K/V projection: the DMA-paced pass (8 PSUM banks) ----
                kaccs = [ps8.tile([128, 512], F32, name="psk", tag="pp")
                         for _ in range(SQC)]
                vaccs = [ps8.tile([128, 512], F32, name="psvt", tag="pp")
                         for _ in range(SQC)]
                for k in range(KC):
                    for qc in range(SQC):
                        nc.tensor.matmul(kaccs[qc], wk_sb[:, k, :],
                                         xt_sb[:, k, qc * 512:(qc + 1) * 512],
                                         start=(k == 0), stop=(k == KC - 1))
                    for sc in range(SQC):
                        nc.tensor.matmul(vaccs[sc], wv_sb[:, k, :],
                                         xt_sb[:, k, sc * 512:(sc + 1) * 512],
                                         start=(k == 0), stop=(k == KC - 1))
                # drain split across ACT and gpsimd so PE isn't gated on one
                # engine's serial copies
                tk = p1tmp.tile([128, S], F16, name="tk", tag="tq")
                for qc in range(SQC):
                    dst = tk[:, qc * 512:(qc + 1) * 512]
                    if qc % 2 == 0:
                        nc.scalar.copy(dst, kaccs[qc])
                    else:
                        nc.vector.tensor_copy(dst, kaccs[qc])
                for sc in range(SQC):
                    dst = vt_sb[:, sc * 512:(sc + 1) * 512]
                    if sc % 2 == 0:
                        nc.scalar.copy(dst, vaccs[sc])
                    else:
                        nc.vector.tensor_copy(dst, vaccs[sc])
                rope_full(kt_sb, tk)

                # ---- Q^T = wq^T @ xT (resident), two 8-bank halves ----
                def q_half(half):
                    accs = []
                    for mh in (2 * half, 2 * half + 1):
                        for qc in range(SQC):
                            psq = ps8.tile([128, 512], F32, name="psq", tag="pp")
                            accs.append((mh, qc, psq))
                    for k in range(KC):
                        for mh, qc, psq in accs:
                            nc.tensor.matmul(psq, wq_sb[:, k, mh * 128:(mh + 1) * 128],
                                             xt_sb[:, k, qc * 512:(qc + 1) * 512],
                                             start=(k == 0), stop=(k == KC - 1))
                    tqs = {}
                    for mh in (2 * half, 2 * half + 1):
                        tqs[mh] = p1tmp.tile([128, S], F16, name="tq", tag="tq")
                    for i, (mh, qc, psq) in enumerate(accs):
                        dst = tqs[mh][:, qc * 512:(qc + 1) * 512]
                        if i % 2 == 0:
                            nc.scalar.copy(dst, psq)
                        else:
                            nc.vector.tensor_copy(dst, psq)
                    for mh in (2 * half, 2 * half + 1):
                        rope_full(qt_sb[:, mh, :], tqs[mh])

                q_half(0)

                # V^T -> V PE-transposes slot between the Q halves (banks
                # cycle here anyway); V is only needed at attention time.
                for gq in range(4):
                    psv = ps8.tile([128, 512], F16, name="psv", tag="pp")
                    for vt in range(4):
                        skt = gq * 4 + vt
                        nc.tensor.matmul(psv[:, vt * 128:(vt + 1) * 128],
                                         vt_sb[:, skt * 128:(skt + 1) * 128],
                                         iden_sb, is_transpose=True,
                                         start=True, stop=True)
                    nc.vector.tensor_copy(
                        v_sb[:, gq * 4:(gq + 1) * 4, :].rearrange("p a b -> p (a b)"),
                        psv)

                q_half(1)

            # ---------------- phase 2+3: attention with fc interleaved ----------------
            # Query blocks descend (qc=3 first): the deepest softmax pipeline
            # runs first, and each block's fc matmuls interleave into the next
            # block's attention to keep PE dense.
            with tc.tile_pool(name="out_sb", bufs=3) as out_sb:

                def fc_block(sqt, split_dma=False):
                    ob = out_sb.tile([128, D], F32, name="ob", tag="ob")
                    for nf in range(4):
                        psf = ps8.tile([128, 512], F32, name="psf", tag="pp")
                        for h2 in range(HG):
                            nc.tensor.matmul(psf,
                                             ctxT[:, h2, sqt * 128:(sqt + 1) * 128],
                                             fcw_sb[:, h2, nf * 512:(nf + 1) * 512],
                                             start=(h2 == 0), stop=(h2 == HG - 1))
                        dst = ob[:, nf * 512:(nf + 1) * 512]
                        nc.vector.tensor_copy(dst, psf)
                        if split_dma:
                            nc.sync.dma_start(
                                out=out[sqt * 128:(sqt + 1) * 128, nf * 512:(nf + 1) * 512],
                                in_=dst)
                    if not split_dma:
                        nc.sync.dma_start(out=out[sqt * 128:(sqt + 1) * 128, :], in_=ob)

                prev_qc = None
                for qc in (0, 1, 2, 3):
                    for h in range(HG):
                        nkc = 4 * (qc + 1)  # causal: sk chunks 0..nkc-1
                        npairs = nkc // 2
                        # PV accumulates in two SEPARATE banks (one group per
                        # bank — same-bank interleaved groups corrupt): the
                        # low column half only needs chunks up to the diagonal
                        psc_a = ps8.tile([128, 256], F32, name="psc_a", tag="pp")
                        psc_b = ps8.tile([128, 256], F32, name="psc_b", tag="pp")
                        psd = ps8.tile([1, 512], F32, name="psd", tag="pp")
                        qs = qt_sb[:, h, qc * 512:(qc + 1) * 512]
                        lim_a = 4 * qc + 1  # last chunk feeding the low half
                        es_tiles = [None] * nkc

                        def scores(kc):
                            t = kc - 4 * qc
                            pss = ps8.tile([128, 512], F32, name="pss", tag="pp")
                            es = es_pool.tile([128, 512], F16, name="es", tag="es")
                            z = 128 * t if t > 0 else 0  # dead columns on diag tiles
                            if z:
                                nc.vector.memset(es[:, 0:z], 0.0)
                            nc.tensor.matmul(pss[:, z:512], kt_sb[:, kc * 128:(kc + 1) * 128],
                                             qs[:, z:512], start=True, stop=True)
                            nc.scalar.activation(es[:, z:512], pss[:, z:512],
                                                 mybir.ActivationFunctionType.Exp,
                                                 scale=SCALE)
                            if t >= 0:
                                # only the 128-wide diagonal strip is partial;
                                # columns beyond it are fully alive
                                nc.vector.tensor_tensor(es[:, z:z + 128], es[:, z:z + 128],
                                                        tri_sb,
                                                        op=mybir.AluOpType.mult)
                            es_tiles[kc] = es

                        def accum_pv(kc):
                            if kc <= lim_a:
                                nc.tensor.matmul(psc_a, v_sb[:, kc, :],
                                                 es_tiles[kc][:, 0:256],
                                                 start=(kc == 0), stop=(kc == lim_a))
                            nc.tensor.matmul(psc_b, v_sb[:, kc, :],
                                             es_tiles[kc][:, 256:512],
                                             start=(kc == 0), stop=(kc == nkc - 1))

                        # softmax denominator: pair adds (+ quad adds when the
                        # block is deep) on DVE, ones-matmul per group on PE,
                        # lagging two groups behind the adds
                        use_quads = nkc >= 8
                        n_group = nkc // 4 if use_quads else npairs
                        pairs = []
                        group = []

                        def accum_den_emit(p):
                            esum = es_pool.tile([128, 512], F16, name="esum", tag="esum")
                            nc.vector.tensor_tensor(esum, es_tiles[2 * p],
                                                    es_tiles[2 * p + 1],
                                                    op=mybir.AluOpType.add)
                            pairs.append(esum)
                            if not use_quads:
                                group.append(esum)
                            elif len(pairs) % 2 == 0:
                                eq = es_pool.tile([128, 512], F16, name="equad", tag="equad")
                                nc.vector.tensor_tensor(eq, pairs[-2], pairs[-1],
                                                        op=mybir.AluOpType.add)
                                group.append(eq)

                        def den_mm(r):
                            nc.tensor.matmul(psd, ones_sb, group[r],
                                             start=(r == 0), stop=(r == n_group - 1))

                        den_issued = 0
                        scores(0)
                        scores(1)
                        for p in range(npairs):
                            if p + 1 < npairs:
                                scores(2 * p + 2)
                                scores(2 * p + 3)
                            accum_pv(2 * p)
                            accum_pv(2 * p + 1)
                            accum_den_emit(p)
                            while den_issued < len(group) - 2:
                                den_mm(den_issued)
                                den_issued += 1
                        while den_issued < n_group:
                            den_mm(den_issued)
                            den_issued += 1

                        rec = nrm_pool.tile([1, 512], F32, name="rec", tag="rec")
                        nc.vector.reciprocal_approx_fast(rec, psd)
                        rb = nrm_pool.tile([128, 512], F32, name="rb", tag="rb")
                        nc.gpsimd.partition_broadcast(rb, rec)
                        nc.vector.tensor_tensor(ctxT[:, h, qc * 512:qc * 512 + 256],
                                                psc_a, rb[:, 0:256], op=mybir.AluOpType.mult)
                        nc.vector.tensor_tensor(ctxT[:, h, qc * 512 + 256:(qc + 1) * 512],
                                                psc_b, rb[:, 256:512], op=mybir.AluOpType.mult)

                        if prev_qc is not None:
                            fc_block(prev_qc * 4 + h)
                    prev_qc = qc

                for j in range(4):
                    fc_block(12 + j, split_dma=(j >= 2))

            nrm_pool.release()
            es_pool.release()
            ps8.release()

    nc.compile()
    return nc


def _get_compiled():
    global _COMPILED
    if _COMPILED is None:
        _COMPILED = _build()
    return _COMPILED


def _prep_inputs(x, w_q, w_kv, fc_w, fc_b, freqs_cos, freqs_sin):
    x = np.asarray(x, dtype=np.float32)
    w_q = np.asarray(w_q, dtype=np.float32)
    w_kv = np.asarray(w_kv, dtype=np.float32)
    fc_w = np.asarray(fc_w, dtype=np.float32)
    freqs_cos = np.asarray(freqs_cos, dtype=np.float32)
    freqs_sin = np.asarray(freqs_sin, dtype=np.float32)

    # rope pair permutation: evens then odds within each head's DK block
    perm = np.concatenate([np.arange(0, DK, 2), np.arange(1, DK, 2)])

    cosT = np.ascontiguousarray(freqs_cos.T).astype(np.float16)  # [64, S]
    sinT = np.ascontiguousarray(freqs_sin.T).astype(np.float16)
    cosT = np.concatenate([cosT, cosT], axis=0)  # duplicate across halves
    sinT = np.concatenate([sinT, sinT], axis=0)

    # tri[i, j] = 1 if i <= j (diagonal-strip causal mask)
    tri = (np.arange(128)[:, None] <= np.arange(128)[None, :]).astype(np.float16)
    onesc = np.ones((128, 1), dtype=np.float16)
    iden = np.eye(128, dtype=np.float16)

    in_maps = []
    for c in range(8):
        b, g = divmod(c, 4)
        xT = np.ascontiguousarray(x[b].T).astype(np.float16)
        wq_g = w_q[:, g * HG * DK:(g + 1) * HG * DK].reshape(D, HG, DK)[:, :, perm]
        wq_g = np.ascontiguousarray(wq_g.reshape(D, HG * DK)).astype(np.float16)
        wk_g = np.ascontiguousarray(w_kv[:, g * DK:(g + 1) * DK][:, perm]).astype(np.float16)
        wv_g = np.ascontiguousarray(w_kv[:, HKV * DK + g * DK:HKV * DK + (g + 1) * DK]).astype(np.float16)
        fcw_g = np.ascontiguousarray(fc_w[g * HG * DK:(g + 1) * HG * DK, :]).astype(np.float16)
        in_maps.append({
            "xT": xT, "wq": wq_g, "wk": wk_g, "wv": wv_g, "fcw": fcw_g,
            "cosT": cosT, "sinT": sinT, "tri": tri, "onesc": onesc,
            "iden": iden,
        })
    return in_maps


_WARMED = False


def kernel_run(trace=False, warmup=True, **inputs):
    global _WARMED
    nc = _get_compiled()
    in_maps = _prep_inputs(**inputs)
    if warmup and not _WARMED:
        # first post-compile execution on a cold device is ~15% slower
        # (table loads / HAM state); do a throwaway run
        run_bass_kernel_spmd(nc, in_maps, core_ids=list(range(8)), trace=False)
        _WARMED = True
    res = run_bass_kernel_spmd(nc, in_maps, core_ids=list(range(8)), trace=trace)
    fc_b = np.asarray(inputs["fc_b"], dtype=np.float32)
    out = np.zeros((B, S, D), dtype=np.float32)
    for c in range(8):
        b = c // 4
        out[b] += res.results[c]["out"]
    out += fc_b[None, None, :]
    return out, res


def kernel(**inputs):
    out, _ = kernel_run(trace=False, **inputs)
    return out


# revision 36
# speedup vs baseline: 1.0473x; 1.0342x over previous
"""GQA multi-head attention (B=2, S=2048, D=2048, HQ=16, HKV=4, DK=128) with
RoPE + causal softmax + output projection, sharded over 8 NeuronCores as
(batch x kv-head-group): core c handles batch c//4, kv head c%4 (4 query
heads). w_q/w_kv column-sharded, fc row-sharded; partial fc outputs are
summed on the host (the "all-reduce").

Schedule: K/V projections run first as the xT-DMA-paced pass (their PE
appetite matches chunk arrival), Q projection halves then run from
SBUF-resident xT at full speed. Attention processes query blocks in
descending size with the previous block's fc matmuls interleaved.
"""

import sys

for _p in ("/opt/trn_rl_repo", "/root/.axon_site", "/root/.axon_site/_ro/trn_rl_repo"):
    if _p not in sys.path:
        sys.path.insert(0, _p)

import numpy as np

import concourse.bass as bass
import concourse.mybir as mybir
import concourse.tile as tile
from concourse import bacc
from concourse.bass_utils import run_bass_kernel_spmd

F32 = mybir.dt.float32
F16 = mybir.dt.float16

B, S, D = 2, 2048, 2048
HKV, NREP, DK = 4, 4, 128
HG = NREP  # query heads per core
KC = D // 128  # contraction chunks
SQC = S // 512  # 512-wide query column chunks
SCALE = float(1.0 / np.sqrt(DK))

_COMPILED = None


def _build():
    nc = bacc.Bacc(None, target_bir_lowering=False, debug=False)

    xT = nc.dram_tensor("xT", [D, S], F16, kind="ExternalInput")
    wq = nc.dram_tensor("wq", [D, HG * DK], F16, kind="ExternalInput")
    wk = nc.dram_tensor("wk", [D, DK], F16, kind="ExternalInput")
    wv = nc.dram_tensor("wv", [D, DK], F16, kind="ExternalInput")
    fcw = nc.dram_tensor("fcw", [HG * DK, D], F16, kind="ExternalInput")
    cosT = nc.dram_tensor("cosT", [128, S], F16, kind="ExternalInput")
    sinT = nc.dram_tensor("sinT", [128, S], F16, kind="ExternalInput")
    tri = nc.dram_tensor("tri", [128, 128], F16, kind="ExternalInput")
    onesc = nc.dram_tensor("onesc", [128, 1], F16, kind="ExternalInput")
    iden = nc.dram_tensor("iden", [128, 128], F16, kind="ExternalInput")
    out = nc.dram_tensor("out", [S, D], F32, kind="ExternalOutput")

    with tile.TileContext(nc) as tc:
        with tc.tile_pool(name="persist", bufs=1) as persist:
            # attention-phase residents
            qt_sb = persist.tile([128, HG, S], F16)  # Q^T, rope'd, per head
            kt_sb = persist.tile([128, S], F16)  # K^T rope'd
            v_sb = persist.tile([128, KC, DK], F16)  # V  [sk, dk] chunks
            ctxT = persist.tile([128, HG, S], F16)  # (softmax @ V)^T per head
            cos_sb = persist.tile([128, S], F16)
            sin_sb = persist.tile([128, S], F16)
            tri_sb = persist.tile([128, 128], F16)
            ones_sb = persist.tile([128, 1], F16)
            iden_sb = persist.tile([128, 128], F16)
            fcw_sb = persist.tile([128, HG, D], F16)

            # pools shared across all phases (no release/realloc barriers)
            ps8 = tc.alloc_tile_pool(name="ps8", bufs=8, space="PSUM")
            es_pool = tc.alloc_tile_pool(name="es_pool", bufs=5)
            nrm_pool = tc.alloc_tile_pool(name="nrm_pool", bufs=3)

            with tc.tile_pool(name="p1sb", bufs=1) as p1sb, \
                 tc.tile_pool(name="p1tmp", bufs=2) as p1tmp:
                xt_sb = p1sb.tile([128, KC, S], F16)
                wq_sb = p1sb.tile([128, KC, HG * DK], F16)
                wk_sb = p1sb.tile([128, KC, DK], F16)
                wv_sb = p1sb.tile([128, KC, DK], F16)
                vt_sb = p1sb.tile([128, S], F16)

                # DMA priority order. sync ring: the 16 xT chunks (the
                # critical stream — K/V projection consumes them at DMA
                # pace). scalar ring: wk/wv first (needed by the paced
                # pass), then wq quarters (Q passes start after chunk 15),
                # then everything needed later still.
                xr = xT.rearrange("(k p) s -> p k s", p=128)
                # chunk 0 lands in two pieces so the first K/V matmuls can
                # start ~1.5us earlier
                nc.sync.dma_start(out=xt_sb[:, 0, 0:1024], in_=xr[:, 0, 0:1024])
                nc.sync.dma_start(out=xt_sb[:, 0, 1024:2048], in_=xr[:, 0, 1024:2048])
                for k in range(1, KC):
                    nc.sync.dma_start(out=xt_sb[:, k, :], in_=xr[:, k, :])
                wkr = wk.rearrange("(k p) m -> p k m", p=128)
                wvr = wv.rearrange("(k p) m -> p k m", p=128)
                wqr = wq.rearrange("(k p) m -> p k m", p=128)
                # scalar ring carries only what the paced pass needs early
                # (wk/wv quarters + the first wq quarter + tiny constants);
                # everything needed later queues on the SYNC ring BEHIND the
                # 16 xT chunks — per-queue FIFO hard-orders those transfers
                # after xT so they can't steal the early HBM window.
                for q4 in range(4):
                    ks = slice(4 * q4, 4 * (q4 + 1))
                    nc.scalar.dma_start(out=wk_sb[:, ks, :], in_=wkr[:, ks, :])
                    nc.scalar.dma_start(out=wv_sb[:, ks, :], in_=wvr[:, ks, :])
                nc.scalar.dma_start(out=wq_sb[:, 0:4, :], in_=wqr[:, 0:4, :])
                nc.scalar.dma_start(out=iden_sb, in_=iden[:])
                nc.scalar.dma_start(out=tri_sb, in_=tri[:])
                nc.scalar.dma_start(out=ones_sb, in_=onesc[:])
                nc.sync.dma_start(out=cos_sb, in_=cosT[:])
                nc.sync.dma_start(out=sin_sb, in_=sinT[:])
                for q4 in range(1, 4):
                    ks = slice(4 * q4, 4 * (q4 + 1))
                    nc.sync.dma_start(out=wq_sb[:, ks, :], in_=wqr[:, ks, :])
                nc.sync.dma_start(out=fcw_sb, in_=fcw.rearrange("(h p) n -> p h n", p=128))

                def rope_full(dst, tq):
                    # dst/tq: [128, S] fp16; evens in partitions 0:64, odds 64:128.
                    # cos/sin are duplicated across both halves so every
                    # SBUF*SBUF tensor op has equal input base partitions.
                    pe, po = tq[0:64, :], tq[64:128, :]
                    t1 = p1tmp.tile([64, S], F16, name="t1", tag="t1")
                    t2 = p1tmp.tile([64, S], F16, name="t2", tag="t2")
                    nc.vector.tensor_tensor(t1, pe, cos_sb[0:64, :], op=mybir.AluOpType.mult)
                    nc.vector.tensor_tensor(t2, po, sin_sb[64:128, :], op=mybir.AluOpType.mult)
                    nc.vector.tensor_tensor(dst[0:64, :], t1, t2, op=mybir.AluOpType.subtract)
                    t3 = p1tmp.tile([64, S], F16, name="t3", tag="t1")
                    t4 = p1tmp.tile([64, S], F16, name="t4", tag="t2")
                    nc.vector.tensor_tensor(t3, pe, sin_sb[0:64, :], op=mybir.AluOpType.mult)
                    nc.vector.tensor_tensor(t4, po, cos_sb[64:128, :], op=mybir.AluOpType.mult)
                    nc.vector.tensor_tensor(dst[64:128, :], t3, t4, op=mybir.AluOpType.add)

                # ---- K/V projection: the DMA-paced pass (8 PSUM banks) ----
                kaccs = [ps8.tile([128, 512], F32, name="psk", tag="pp")
                         for _ in range(SQC)]
                vaccs = [ps8.tile([128, 512], F32, name="psvt", tag="pp")
                         for _ in range(SQC)]
                for k in range(KC):
                    for qc in range(SQC):
                        nc.tensor.matmul(kaccs[qc], wk_sb[:, k, :],
                                         xt_sb[:, k, qc * 512:(qc + 1) * 512],
                                         start=(k == 0), stop=(k == KC - 1))
                    for sc in range(SQC):
                        nc.tensor.matmul(vaccs[sc], wv_sb[:, k, :],
                                         xt_sb[:, k, sc * 512:(sc + 1) * 512],
                                         start=(k == 0), stop=(k == KC - 1))
                # drain split across ACT and gpsimd so PE isn't gated on one
                # engine's serial copies
                tk = p1tmp.tile([128, S], F16, name="tk", tag="tq")
                for qc in range(SQC):
                    dst = tk[:, qc * 512:(qc + 1) * 512]
                    if qc % 2 == 0:
                        nc.scalar.copy(dst, kaccs[qc])
                    else:
                        nc.vector.tensor_copy(dst, kaccs[qc])
                for sc in range(SQC):
                    dst = vt_sb[:, sc * 512:(sc + 1) * 512]
                    if sc % 2 == 0:
                        nc.scalar.copy(dst, vaccs[sc])
                    else:
                        nc.vector.tensor_copy(dst, vaccs[sc])
                rope_full(kt_sb, tk)

                # ---- Q^T = wq^T @ xT (resident), two 8-bank halves ----
                def q_half(half):
                    accs = []
                    for mh in (2 * half, 2 * half + 1):
                        for qc in range(SQC):
                            psq = ps8.tile([128, 512], F32, name="psq", tag="pp")
                            accs.append((mh, qc, psq))
                    for k in range(KC):
                        for mh, qc, psq in accs:
                            nc.tensor.matmul(psq, wq_sb[:, k, mh * 128:(mh + 1) * 128],
                                             xt_sb[:, k, qc * 512:(qc + 1) * 512],
                                             start=(k == 0), stop=(k == KC - 1))
                    tqs = {}
                    for mh in (2 * half, 2 * half + 1):
                        tqs[mh] = p1tmp.tile([128, S], F16, name="tq", tag="tq")
                    for i, (mh, qc, psq) in enumerate(accs):
                        dst = tqs[mh][:, qc * 512:(qc + 1) * 512]
                        if i % 2 == 0:
                            nc.scalar.copy(dst, psq)
                        else:
                            nc.vector.tensor_copy(dst, psq)
                    for mh in (2 * half, 2 * half + 1):
                        rope_full(qt_sb[:, mh, :], tqs[mh])

                q_half(0)

                # V^T -> V PE-transposes slot between the Q halves (banks
                # cycle here anyway); V is only needed at attention time.
                for gq in range(4):
                    psv = ps8.tile([128, 512], F16, name="psv", tag="pp")
                    for vt in range(4):
                        skt = gq * 4 + vt
                        nc.tensor.matmul(psv[:, vt * 128:(vt + 1) * 128],
                                         vt_sb[:, skt * 128:(skt + 1) * 128],
                                         iden_sb, is_transpose=True,
                                         start=True, stop=True)
                    nc.vector.tensor_copy(
                        v_sb[:, gq * 4:(gq + 1) * 4, :].rearrange("p a b -> p (a b)"),
                        psv)

                q_half(1)

            # ---------------- phase 2+3: attention with fc interleaved ----------------
            # Query blocks descend (qc=3 first): the deepest softmax pipeline
            # runs first, and each block's fc matmuls interleave into the next
            # block's attention to keep PE dense.
            with tc.tile_pool(name="out_sb", bufs=3) as out_sb:

                def fc_block(sqt, split_dma=False):
                    ob = out_sb.tile([128, D], F32, name="ob", tag="ob")
                    for nf in range(4):
                        psf = ps8.tile([128, 512], F32, name="psf", tag="pp")
                        for h2 in range(HG):
                            nc.tensor.matmul(psf,
                                             ctxT[:, h2, sqt * 128:(sqt + 1) * 128],
                                             fcw_sb[:, h2, nf * 512:(nf + 1) * 512],
                                             start=(h2 == 0), stop=(h2 == HG - 1))
                        dst = ob[:, nf * 512:(nf + 1) * 512]
                        nc.vector.tensor_copy(dst, psf)
                        if split_dma:
                            nc.sync.dma_start(
                                out=out[sqt * 128:(sqt + 1) * 128, nf * 512:(nf + 1) * 512],
                                in_=dst)
                    if not split_dma:
                        nc.sync.dma_start(out=out[sqt * 128:(sqt + 1) * 128, :], in_=ob)

                prev_qc = None
                for qc in (0, 1, 2, 3):
                    for h in range(HG):
                        nkc = 4 * (qc + 1)  # causal: sk chunks 0..nkc-1
                        npairs = nkc // 2
                        # PV accumulates in two SEPARATE banks (one group per
                        # bank — same-bank interleaved groups corrupt): the
                        # low column half only needs chunks up to the diagonal
                        psc_a = ps8.tile([128, 256], F32, name="psc_a", tag="pp")
                        psc_b = ps8.tile([128, 256], F32, name="psc_b", tag="pp")
                        psd = ps8.tile([1, 512], F32, name="psd", tag="pp")
                        qs = qt_sb[:, h, qc * 512:(qc + 1) * 512]
                        lim_a = 4 * qc + 1  # last chunk feeding the low half
                        es_tiles = [None] * nkc

                        def scores(kc):
                            t = kc - 4 * qc
                            pss = ps8.tile([128, 512], F32, name="pss", tag="pp")
                            es = es_pool.tile([128, 512], F16, name="es", tag="es")
                            z = 128 * t if t > 0 else 0  # dead columns on diag tiles
                            if z:
                                nc.vector.memset(es[:, 0:z], 0.0)
                            nc.tensor.matmul(pss[:, z:512], kt_sb[:, kc * 128:(kc + 1) * 128],
                                             qs[:, z:512], start=True, stop=True)
                            nc.scalar.activation(es[:, z:512], pss[:, z:512],
                                                 mybir.ActivationFunctionType.Exp,
                                                 scale=SCALE)
                            if t >= 0:
                                # only the 128-wide diagonal strip is partial;
                                # columns beyond it are fully alive
                                nc.vector.tensor_tensor(es[:, z:z + 128], es[:, z:z + 128],
                                                        tri_sb,
                                                        op=mybir.AluOpType.mult)
                            es_tiles[kc] = es

                        def accum_pv(kc):
                            if kc <= lim_a:
                                nc.tensor.matmul(psc_a, v_sb[:, kc, :],
                                                 es_tiles[kc][:, 0:256],
                                                 start=(kc == 0), stop=(kc == lim_a))
                            nc.tensor.matmul(psc_b, v_sb[:, kc, :],
                                             es_tiles[kc][:, 256:512],
                                             start=(kc == 0), stop=(kc == nkc - 1))

                        # softmax denominator: pair adds (+ quad adds when the
                        # block is deep) on DVE, ones-matmul per group on PE,
                        # lagging two groups behind the adds
                        use_quads = nkc >= 8
                        n_group = nkc // 4 if use_quads else npairs
                        pairs = []
                        group = []

                        def accum_den_emit(p):
                            esum = es_pool.tile([128, 512], F16, name="esum", tag="esum")
                            nc.vector.tensor_tensor(esum, es_tiles[2 * p],
                                                    es_tiles[2 * p + 1],
                                                    op=mybir.AluOpType.add)
                            pairs.append(esum)
                            if not use_quads:
                                group.append(esum)
                            elif len(pairs) % 2 == 0:
                                eq = es_pool.tile([128, 512], F16, name="equad", tag="equad")
                                nc.vector.tensor_tensor(eq, pairs[-2], pairs[-1],
                                                        op=mybir.AluOpType.add)
                                group.append(eq)

                        def den_mm(r):
                            nc.tensor.matmul(psd, ones_sb, group[r],
                                             start=(r == 0), stop=(r == n_group - 1))

                        den_issued = 0
                        scores(0)
                        scores(1)
                        for p in range(npairs):
                            if p + 1 < npairs:
                                scores(2 * p + 2)
                                scores(2 * p + 3)
                            accum_pv(2 * p)
                            accum_pv(2 * p + 1)
                            accum_den_emit(p)
                            while den_issued < len(group) - 2:
                                den_mm(den_issued)
                                den_issued += 1
                        while den_issued < n_group:
                            den_mm(den_issued)
                            den_issued += 1

                        rec = nrm_pool.tile([1, 512], F32, name="rec", tag="rec")
                        nc.vector.reciprocal_approx_fast(rec, psd)
                        rb = nrm_pool.tile([128, 512], F32, name="rb", tag="rb")
                        nc.gpsimd.partition_broadcast(rb, rec)
                        nc.vector.tensor_tensor(ctxT[:, h, qc * 512:qc * 512 + 256],
                                                psc_a, rb[:, 0:256], op=mybir.AluOpType.mult)
                        nc.vector.tensor_tensor(ctxT[:, h, qc * 512 + 256:(qc + 1) * 512],
                                                psc_b, rb[:, 256:512], op=mybir.AluOpType.mult)

                        if prev_qc is not None:
                            fc_block(prev_qc * 4 + h)
                    prev_qc = qc

                for j in range(4):
                    fc_block(12 + j, split_dma=(j >= 2))

            nrm_pool.release()
            es_pool.release()
            ps8.release()

    nc.compile()
    return nc


def _get_compiled():
    global _COMPILED
    if _COMPILED is None:
        _COMPILED = _build()
    return _COMPILED


def _prep_inputs(x, w_q, w_kv, fc_w, fc_b, freqs_cos, freqs_sin):
    x = np.asarray(x, dtype=np.float32)
    w_q = np.asarray(w_q, dtype=np.float32)
    w_kv = np.asarray(w_kv, dtype=np.float32)
    fc_w = np.asarray(fc_w, dtype=np.float32)
    freqs_cos = np.asarray(freqs_cos, dtype=np.float32)
    freqs_sin = np.asarray(freqs_sin, dtype=np.float32)

    # rope pair permutation: evens then odds within each head's DK block
    perm = np.concatenate([np.arange(0, DK, 2), np.arange(1, DK, 2)])

    cosT = np.ascontiguousarray(freqs_cos.T).astype(np.float16)  # [64, S]
    sinT = np.ascontiguousarray(freqs_sin.T).astype(np.float16)
    cosT = np.concatenate([cosT, cosT], axis=0)  # duplicate across halves
    sinT = np.concatenate([sinT, sinT], axis=0)

    # tri[i, j] = 1 if i <= j (diagonal-strip causal mask)
    tri = (np.arange(128)[:, None] <= np.arange(128)[None, :]).astype(np.float16)
    onesc = np.ones((128, 1), dtype=np.float16)
    iden = np.eye(128, dtype=np.float16)

    in_maps = []
    for c in range(8):
        b, g = divmod(c, 4)
        xT = np.ascontiguousarray(x[b].T).astype(np.float16)
        wq_g = w_q[:, g * HG * DK:(g + 1) * HG * DK].reshape(D, HG, DK)[:, :, perm]
        wq_g = np.ascontiguousarray(wq_g.reshape(D, HG * DK)).astype(np.float16)
        wk_g = np.ascontiguousarray(w_kv[:, g * DK:(g + 1) * DK][:, perm]).astype(np.float16)
        wv_g = np.ascontiguousarray(w_kv[:, HKV * DK + g * DK:HKV * DK + (g + 1) * DK]).astype(np.float16)
        fcw_g = np.ascontiguousarray(fc_w[g * HG * DK:(g + 1) * HG * DK, :]).astype(np.float16)
        in_maps.append({
            "xT": xT, "wq": wq_g, "wk": wk_g, "wv": wv_g, "fcw": fcw_g,
            "cosT": cosT, "sinT": sinT, "tri": tri, "onesc": onesc,
            "iden": iden,
        })
    return in_maps


_WARMED = False


def kernel_run(trace=False, warmup=True, **inputs):
    global _WARMED
    nc = _get_compiled()
    in_maps = _prep_inputs(**inputs)
    if warmup and not _WARMED:
        # first post-compile execution on a cold device is ~15% slower
        # (table loads / HAM state); do a throwaway run
        run_bass_kernel_spmd(nc, in_maps, core_ids=list(range(8)), trace=False)
        _WARMED = True
    res = run_bass_kernel_spmd(nc, in_maps, core_ids=list(range(8)), trace=trace)
    fc_b = np.asarray(inputs["fc_b"], dtype=np.float32)
    out = np.zeros((B, S, D), dtype=np.float32)
    for c in range(8):
        b = c // 4
        out[b] += res.results[c]["out"]
    out += fc_b[None, None, :]
    return out, res


def kernel(**inputs):
    out, _ = kernel_run(trace=False, **inputs)
    return out
